# revision 4
# baseline (speedup 1.0000x reference)
"""Trainium2 Bass kernel for the ConcreteLayer training forward pass.

Computes out = x @ softmax((weight - ln(-ln((1-tiny)*uniform + tiny))) / T, axis=1)

Strategy (8 NeuronCores):
  - Softmax is row-sharded: core c computes softmax for weight rows
    [c*512, (c+1)*512) over ALL 1024 columns, so the softmax (axis=1)
    is fully local - no partial-sum exchange.  Normalized samples are
    exchanged with 4 staged 8-rank AllGathers (256 KB bf16 per core per
    stage) so the GEMM can start after the first stage lands.
  - GEMM is batch-sharded: core c computes out rows [c*512, (c+1)*512)
    = xT_slice.T @ samples, accumulated over all 32 k-tiles in PSUM
    as AllGather stages arrive.  x is converted to bf16 on the host.
"""

import sys

import numpy as np

for _p in ("/opt/trn_rl_repo",):
    if _p not in sys.path:
        sys.path.insert(0, _p)

B, IN, OUT = 4096, 4096, 1024
NCORES = 8
BS = B // NCORES  # 512 batch rows per core
RS = IN // NCORES  # 512 softmax rows per core
P = 128
LKT = RS // P  # 4 local softmax row tiles (= AllGather stages)
KT = IN // P  # 32 contraction tiles
MBT = BS // P  # 4 output row tiles per core
NH = OUT // 512  # moving-dim chunks per matmul rhs
TINY = float(np.finfo(np.float32).tiny)

_PROGRAM = None
LAST_RESULT = None


def _pin_act_tables():
    """Steer the act-table-load pass to one set (has both Ln and Exp) so the
    compiler emits one ACT_TABLE_LOAD instead of reloading per tile."""
    import concourse.mybir as mybir
    from concourse import bacc, hw_specs

    orig = hw_specs.get_activation_tables.__wrapped__
    target = "natural_log_exp_and_others"
    strip = {
        mybir.ActivationFunctionType.Ln,
        mybir.ActivationFunctionType.Exp,
    }

    def pinned(arch):
        tables = orig(arch)
        if target not in tables:
            return tables
        return {
            name: (set(fns) if name == target else {f for f in fns if f not in strip})
            for name, fns in tables.items()
        }

    bacc.get_activation_tables = pinned


def _build_program():
    import concourse.bass as bass
    import concourse.mybir as mybir
    import concourse.tile as tile
    from concourse import bacc
    from contextlib import ExitStack

    _pin_act_tables()

    f32 = mybir.dt.float32
    bf16 = mybir.dt.bfloat16
    Ln = mybir.ActivationFunctionType.Ln
    Exp = mybir.ActivationFunctionType.Exp

    nc = bacc.Bacc(
        "TRN2", target_bir_lowering=False, debug=False, num_devices=NCORES
    )

    xt_d = nc.dram_tensor("xt", [IN, BS], bf16, kind="ExternalInput")
    wh_d = nc.dram_tensor("wh", [RS, OUT], f32, kind="ExternalInput")
    uh_d = nc.dram_tensor("uh", [RS, OUT], f32, kind="ExternalInput")
    t_d = nc.dram_tensor("tt", [1], f32, kind="ExternalInput")
    out_d = nc.dram_tensor("out", [BS, OUT], f32, kind="ExternalOutput")

    replica_groups = [[0, 1, 2, 3, 4, 5, 6, 7]]

    with tile.TileContext(nc) as tc, ExitStack() as ctx:
        dram = ctx.enter_context(tc.tile_pool(name="dram", bufs=1, space="DRAM"))
        singles = ctx.enter_context(tc.tile_pool(name="singles", bufs=1))
        chunks = ctx.enter_context(tc.tile_pool(name="chunks", bufs=2))
        outp = ctx.enter_context(tc.tile_pool(name="outp", bufs=2))
        psum = ctx.enter_context(tc.tile_pool(name="psum", bufs=1, space="PSUM"))

        # 1/T broadcast to all partitions.
        t_sb = singles.tile([P, 1], f32)
        t_ap = t_d.ap()
        nc.sync.dma_start(
            out=t_sb, in_=bass.AP(tensor=t_ap.tensor, offset=0, ap=[[0, P], [1, 1]])
        )
        invt = singles.tile([P, 1], f32)
        nc.vector.reciprocal(invt, t_sb)

        zero_t = singles.tile([P, 1], f32)
        nc.vector.memset(zero_t, 0.0)
        tiny_t = singles.tile([P, 1], f32)
        nc.vector.memset(tiny_t, TINY)

        # Resident operands: all 32 k-tiles of xT (bf16, loaded up front);
        # the gathered samples tile e_all is declared below.
        xt_all = singles.tile([P, KT, BS], bf16)

        # Prefetch xT on the gpsimd (SWDGE) queue, split in two for overlap.
        HG = KT // 2
        for half in range(2):
            src = xt_d[half * HG * P : (half + 1) * HG * P, :].rearrange(
                "(g p) b -> p g b", p=P
            )
            nc.gpsimd.dma_start(out=xt_all[:, half * HG : (half + 1) * HG, :], in_=src)

        NAG = 2  # number of AllGather rounds
        SPG = LKT // NAG  # local row tiles per round
        cc_in = [
            dram.tile([SPG * P, OUT], bf16, name=f"cc_in{a}", tag=f"cc_in{a}")
            for a in range(NAG)
        ]
        cc_out = [
            dram.tile(
                [NCORES * SPG * P, OUT],
                bf16,
                name=f"cc_out{a}",
                tag=f"cc_out{a}",
                addr_space="Shared",
            )
            for a in range(NAG)
        ]

        # All u/w loads issued up front so the sync DMA stream never blocks
        # on downstream compute.
        u_ts, w_ts = [], []
        for s in range(LKT):
            u_t = chunks.tile([P, OUT], f32, tag=f"u{s}", name="u_t", bufs=1)
            w_t = chunks.tile([P, OUT], f32, tag=f"w{s}", name="w_t", bufs=1)
            nc.sync.dma_start(out=u_t, in_=uh_d[s * P : (s + 1) * P, :])
            nc.sync.dma_start(out=w_t, in_=wh_d[s * P : (s + 1) * P, :])
            u_ts.append(u_t)
            w_ts.append(w_t)

        def softmax_stage(s):
            # Local rows [s*128, (s+1)*128): full softmax over 1024 cols.
            u_t, w_t = u_ts[s], w_ts[s]
            # v = ln((1 - tiny)*u + tiny)            (negative)
            nc.scalar.activation(u_t, u_t, Ln, bias=tiny_t[:], scale=1.0 - TINY)
            # m = ln(-v) = -gumbel
            nc.scalar.activation(u_t, u_t, Ln, bias=zero_t[:], scale=-1.0)
            # d = w - m = w + gumbel
            nc.vector.tensor_sub(u_t, w_t, u_t)
            # e = exp(d / T) with per-row sum over all 1024 cols.
            e_st = chunks.tile([P, OUT], bf16, tag="e", name="e_st")
            sums = chunks.tile([P, 1], f32, tag="sums", name="sums")
            nc.scalar.activation(
                e_st, u_t, Exp, bias=zero_t[:], scale=invt[:], accum_out=sums
            )
            rsum = chunks.tile([P, 1], f32, tag="rsum", name="rsum")
            nc.vector.reciprocal(rsum, sums)
            nc.vector.tensor_scalar_mul(e_st, e_st, rsum)
            return e_st

        e_sts = [softmax_stage(s) for s in range(LKT)]
        # Normalized stages to the collective input buffers (after all wu
        # loads in the sync stream, so those were never blocked).
        for s in range(LKT):
            a, s2 = s // SPG, s % SPG
            nc.sync.dma_start(
                out=cc_in[a][s2 * P : (s2 + 1) * P, :], in_=e_sts[s]
            )

        # Sample exchange: round a delivers k-tiles {r*LKT + a*SPG + s2}.
        for a in range(NAG):
            nc.gpsimd.collective_compute(
                "AllGather",
                mybir.AluOpType.bypass,
                replica_groups=replica_groups,
                ins=[cc_in[a].opt()],
                outs=[cc_out[a].opt()],
            )
        # e_all[:, a, q, :] with q = r*SPG + s2  ->  global k-tile
        # g = r*LKT + a*SPG + s2.
        e_all = singles.tile([P, NAG, NCORES * SPG, OUT], bf16)
        for a in range(NAG):
            nc.sync.dma_start(
                out=e_all[:, a, :, :],
                in_=cc_out[a][:].rearrange("(q p) c -> p q c", p=P),
            )

        ps_tiles = [
            psum.tile([P, OUT], f32, tag=f"ps{mb}", name=f"ps{mb}")
            for mb in range(MBT)
        ]

        # GEMM: accumulate over k in round order so PSUM fills as rounds land.
        NQ = NCORES * SPG
        for a in range(NAG):
            for q in range(NQ):
                r, s2 = q // SPG, q % SPG
                g = r * LKT + a * SPG + s2  # global k-tile index
                for mb in range(MBT):
                    for h in range(NH):
                        nc.tensor.matmul(
                            ps_tiles[mb][:, h * 512 : (h + 1) * 512],
                            lhsT=xt_all[:, g, mb * P : (mb + 1) * P],
                            rhs=e_all[:, a, q, h * 512 : (h + 1) * 512],
                            start=(a == 0 and q == 0),
                            stop=(a == NAG - 1 and q == NQ - 1),
                        )

        for mb in range(MBT):
            o_t = outp.tile([P, OUT], f32, tag="o")
            nc.vector.tensor_copy(o_t, ps_tiles[mb][:])
            nc.sync.dma_start(out=out_d[mb * P : (mb + 1) * P, :], in_=o_t)

    nc.compile()
    return nc


def kernel(x, weight, uniform, T):
    global _PROGRAM, LAST_RESULT
    import ml_dtypes
    from concourse.bass_utils import run_bass_kernel_spmd

    if _PROGRAM is None:
        _PROGRAM = _build_program()
    nc = _PROGRAM

    x = np.asarray(x, dtype=np.float32)
    weight = np.ascontiguousarray(np.asarray(weight, dtype=np.float32))
    uniform = np.ascontiguousarray(np.asarray(uniform, dtype=np.float32))
    T = np.ascontiguousarray(np.asarray(T, dtype=np.float32)).reshape([1])

    xt = np.ascontiguousarray(x.T).astype(ml_dtypes.bfloat16)  # [IN, B] bf16
    in_maps = []
    for c in range(NCORES):
        in_maps.append(
            {
                "xt": np.ascontiguousarray(xt[:, c * BS : (c + 1) * BS]),
                "wh": np.ascontiguousarray(weight[c * RS : (c + 1) * RS, :]),
                "uh": np.ascontiguousarray(uniform[c * RS : (c + 1) * RS, :]),
                "tt": T,
            }
        )

    res = run_bass_kernel_spmd(nc, in_maps, core_ids=list(range(NCORES)))
    LAST_RESULT = res

    out = np.empty((B, OUT), dtype=np.float32)
    for c in range(NCORES):
        out[c * BS : (c + 1) * BS, :] = res.results[c]["out"]
    return out
